# revision 13
# baseline (speedup 1.0000x reference)
"""Trainium2 Bass kernel for GQA attention with QK-RMSNorm, partial mRoPE,
causal mask and sigmoid output gate (nn_Attention_70557722739259).

Model dims: B=2, T=2048, D=2048, N=16 Q heads, K=2 KV heads, H=256.
Sharding over 8 NeuronCores: DP=2 over batch x TP=4 over head groups
(4 Q heads + their shared KV head per core). Each core computes a partial
output projection over its 4 heads; the host sums the 4 partials per batch
(Megatron-style unshard).

All device inputs are host-packed into [128, X] layouts so each tensor
loads in 1-4 large DMAs (the DMA queue is the scarce resource, not bytes).

Per-core device algorithm (all matmuls bf16, softmax f32):
  phase 1 (single full-T pass per projection): k/v then per q-head
           projections from host-pre-packed x^T; k/v/q extracted straight
           from PSUM (Pool-engine copies), RMSNorm stats via one DVE
           tensor_tensor_reduce per tile, sigmoid gate on Act from PSUM,
           per-head batched Sqrt for rms, RoPE + norm scale token-major,
           PE-transpose Q,K to feature-major (H, T). Each head's RoPE
           (DVE) overlaps the next head's projection matmuls (PE).
  phase 2: per head, S^T = K^T.T @ Q^T tiles (keys on partitions), exp via
           ACT (1/16 scale folded in; no max subtraction - scores are O(10)),
           diagonal-block masking via Pool affine_select on the first 128
           columns only (fully-masked sub-blocks are never computed: the
           moving operand is narrowed on the diagonal strip), AV + softmax
           denominator accumulated in PSUM via ones-column in V. The i-loop
           is software-pipelined depth 3 (scores for i+3 issue before AV of
           i) so exp latency never stalls PE. Head flushes are split: the
           DVE renorm/gate chain issues right after the AV loop (it runs
           under the interleaved o-proj), the PE transposes are deferred
           past the next head's score prologue.
  phase 3: o-proj partial from gated qkv^T, interleaved into phase 2 with a
           one-block lag to keep PE busy during per-head flushes.
  PSUM: tag "big" (4 bufs: projections, scores, o-proj) + av0..av3
  accumulators (also reused as transpose targets) = exactly 8 banks.
"""
import sys
sys.path.insert(0, "/opt/trn_rl_repo")
import numpy as np
import ml_dtypes

from concourse import bacc, tile, mybir
from concourse import bass_utils
from concourse.masks import make_identity

BF16 = ml_dtypes.bfloat16
F32 = mybir.dt.float32
BF = mybir.dt.bfloat16

B, T, D = 2, 2048, 2048
N_HEADS, N_KV, H = 16, 2, 256
HEADS_PC = 4            # q heads per core (TP=4)
ROPE_THETA = 1000000
ROTARY = 64             # int(H * 0.25)
FREQ = 32
NORM_EPS = 1e-6
K_MASK = -2.3819763e38
SCALE = H ** (-0.5)     # 1/16

TT = T // 128           # 16 token tiles
DC = D // 128           # 16 contraction chunks
PIPE = 4                # phase-2 score/exp software-pipeline depth

LAST_RESULT = None
LAST_IN_MAPS = None
_COMPILED = {}


def _build(mode="causal", apply_w=False):
    nc = bacc.Bacc("TRN2", target_bir_lowering=False, debug=False,
                   enable_asserts=True, num_devices=8)
    Act = mybir.ActivationFunctionType
    Alu = mybir.AluOpType

    # host-packed layouts (see kernel() for the packing):
    #   xT  [128, 32768]: col = ti*2048 + d*128 + t
    #   wq  [128, 32768]: col = h*8192 + d*512 + f
    #   wkv [128, 8192]:  col = d*512 + f                        (f: k 0:256, v 256:512)
    #   wo  [128, 16384]: col = c*2048 + dcol                    (c: 8 feature chunks)
    #   cc/ss [128, 1024]: col = ti*64 + r
    xT = nc.dram_tensor("xT", (128, 16 * 2048), BF, kind="ExternalInput").ap()
    wq = nc.dram_tensor("wq", (128, 16 * 2048), BF, kind="ExternalInput").ap()
    wkv = nc.dram_tensor("wkv", (128, 16 * 512), BF, kind="ExternalInput").ap()
    wo = nc.dram_tensor("wo", (128, 8 * 2048), BF, kind="ExternalInput").ap()
    cc = nc.dram_tensor("cc", (128, TT * ROTARY), BF, kind="ExternalInput").ap()
    ss = nc.dram_tensor("ss", (128, TT * ROTARY), BF, kind="ExternalInput").ap()
    if apply_w:
        qw = nc.dram_tensor("qw", (128, H), F32, kind="ExternalInput").ap()
        kw = nc.dram_tensor("kw", (128, H), F32, kind="ExternalInput").ap()
    if mode == "arbitrary":
        am = nc.dram_tensor("am", (T, T), F32, kind="ExternalInput").ap()
    out = nc.dram_tensor("out", (T, D), BF, kind="ExternalOutput").ap()

    causal = (mode == "causal")

    def xcol(d, ti):
        return ti * 2048 + d * 128

    with tile.TileContext(nc) as tc:
        with tc.tile_pool(name="const", bufs=1) as constp, \
             tc.tile_pool(name="attn", bufs=1) as ap_, \
             tc.tile_pool(name="psum", bufs=1, space="PSUM") as psum:

            # ---- constants ----
            ident = constp.tile([128, 128], BF, tag="ident", name="ident")
            make_identity(nc, ident[:])
            epst = constp.tile([128, 1], F32, tag="epst", name="epst")
            nc.gpsimd.memset(epst[:], NORM_EPS)
            if apply_w:
                qw_sb = constp.tile([128, H], F32, tag="qw", name="qw")
                kw_sb = constp.tile([128, H], F32, tag="kw", name="kw")
                nc.sync.dma_start(qw_sb[:], qw[:])
                nc.sync.dma_start(kw_sb[:], kw[:])

            # ---- persistent attention tensors (span phase 1 -> 2) ----
            QT = [[ap_.tile([128, T], BF, tag=f"QT{h}_{c}", name=f"QT{h}_{c}")
                   for c in range(2)] for h in range(HEADS_PC)]
            KT = [ap_.tile([128, T], BF, tag=f"KT{c}", name=f"KT{c}")
                  for c in range(2)]
            V = [ap_.tile([128, H + 1], BF, tag=f"V{i}", name=f"V{i}")
                 for i in range(TT)]
            for i in range(TT):
                nc.gpsimd.memset(V[i][:, H:H + 1], 1.0)
            gate = [[ap_.tile([128, H], BF, tag=f"g{h}_{i}", name=f"g{h}_{i}")
                     for i in range(TT)] for h in range(HEADS_PC)]

            tp_ctr = [0]            # rotates transpose targets over av0..av3

            def transpose_128(src_ap, dst_ap, nm):
                """PE-transpose a [128,128] bf16 block via an av psum slot."""
                tag = f"av{tp_ctr[0] % 4}"
                tp_ctr[0] += 1
                tp = psum.tile([128, 128], BF, tag=tag, name=f"tp{nm}")
                nc.tensor.transpose(tp[:], src_ap, ident[:])
                nc.any.tensor_copy(dst_ap, tp[:])

            # ================= phase 1 =================
            with tc.tile_pool(name="praw", bufs=1) as praw, \
                 tc.tile_pool(name="proj", bufs=1) as proj, \
                 tc.tile_pool(name="p1c", bufs=3) as p1c:
                # DMA order = need order: wkv, xT[cs0], wq[h0], xT[cs1..3],
                # wq[h1], cc, ss; wq[h2], wq[h3] issued in the head loop.
                xT_sb = proj.tile([128, 16 * 2048], BF, tag="xT", name="xT_sb")
                wkv_sb = proj.tile([128, 16 * 512], BF, tag="wqh", bufs=2,
                                   name="wkv_sb")
                # interleave so the first projection group's inputs land first
                nc.sync.dma_start(wkv_sb[:, 0:2048], wkv[:, 0:2048])
                nc.sync.dma_start(xT_sb[:, 0:2048], xT[:, 0:2048])
                nc.sync.dma_start(wkv_sb[:, 2048:4096], wkv[:, 2048:4096])
                nc.sync.dma_start(xT_sb[:, 2048:4096], xT[:, 2048:4096])
                for q_ in range(2, 4):
                    nc.sync.dma_start(wkv_sb[:, q_ * 2048:(q_ + 1) * 2048],
                                      wkv[:, q_ * 2048:(q_ + 1) * 2048])
                wq_sb = [None] * HEADS_PC

                def load_wq(h):
                    w_ = proj.tile([128, 16 * 512], BF, tag="wqh", bufs=2,
                                   name=f"wq_sb{h}")
                    for q_ in range(2):
                        nc.sync.dma_start(
                            w_[:, q_ * 4096:(q_ + 1) * 4096],
                            wq[:, h * 8192 + q_ * 4096:
                               h * 8192 + (q_ + 1) * 4096])
                    wq_sb[h] = w_

                load_wq(0)
                for q_ in range(2, 16):
                    nc.sync.dma_start(xT_sb[:, q_ * 2048:(q_ + 1) * 2048],
                                      xT[:, q_ * 2048:(q_ + 1) * 2048])
                load_wq(1)
                cc_sb = praw.tile([128, TT * ROTARY], BF, tag="cc", name="cc_sb")
                ss_sb = praw.tile([128, TT * ROTARY], BF, tag="ss", name="ss_sb")
                nc.sync.dma_start(cc_sb[:], cc[:])
                nc.sync.dma_start(ss_sb[:], ss[:])

                # ---- rope + norm scale + transpose for one token tile ----
                def rope_pass(raw, rinv_ap, dst_tiles, ti, w_sb, nm):
                    src = raw
                    if apply_w:
                        srw = p1c.tile([128, H], F32, tag="srw", name=f"srw{nm}")
                        nc.vector.tensor_mul(srw[:], raw[:], w_sb[:])
                        src = srw
                    ccs = cc_sb[:, ti * ROTARY:(ti + 1) * ROTARY]
                    sss = ss_sb[:, ti * ROTARY:(ti + 1) * ROTARY]
                    rot = p1c.tile([128, ROTARY], BF, tag="rot", name=f"rot{nm}")
                    t2 = p1c.tile([128, ROTARY], BF, tag="rot2", name=f"rot2{nm}")
                    nc.vector.tensor_mul(rot[:], src[:, 0:ROTARY], ccs)
                    nc.vector.tensor_mul(t2[:, 0:FREQ], src[:, FREQ:ROTARY],
                                         sss[:, 0:FREQ])
                    nc.vector.tensor_mul(t2[:, FREQ:ROTARY], src[:, 0:FREQ],
                                         sss[:, FREQ:ROTARY])
                    nc.vector.tensor_add(rot[:], rot[:], t2[:])
                    tok = p1c.tile([128, H], BF, tag="tok", name=f"tok{nm}")
                    nc.vector.tensor_scalar_mul(tok[:, 0:ROTARY], rot[:],
                                                rinv_ap)
                    nc.vector.tensor_scalar_mul(tok[:, ROTARY:H],
                                                src[:, ROTARY:H], rinv_ap)
                    for c2 in range(2):
                        transpose_128(tok[:, c2 * 128:(c2 + 1) * 128],
                                      dst_tiles[c2][:, ti * 128:(ti + 1) * 128],
                                      f"{nm}_{c2}")

                # ---- k/v pass over all token tiles ----
                k_raw = [praw.tile([128, H], BF, tag=f"kr{ti}", name=f"kr{ti}")
                         for ti in range(TT)]
                ssqk = praw.tile([128, TT], F32, tag="ssqk", name="ssqk")
                rmsk = praw.tile([128, TT], F32, tag="rmsk", name="rmsk")
                rinvk = praw.tile([128, TT], F32, tag="rinvk", name="rinvk")

                def rms_quarter(ssq, rms_, rinv_, q_):
                    sl = slice(q_ * 4, (q_ + 1) * 4)
                    nc.scalar.activation(rms_[:, sl], ssq[:, sl], Act.Sqrt,
                                         scale=1.0 / H, bias=epst[:])
                    nc.vector.reciprocal(rinv_[:, sl], rms_[:, sl])

                for ti in range(TT):
                    pk = psum.tile([128, 2 * H], F32, tag="big", bufs=4,
                                   name=f"pk{ti}")
                    for d in range(DC):
                        nc.tensor.matmul(
                            pk[:], xT_sb[:, xcol(d, ti):xcol(d, ti) + 128],
                            wkv_sb[:, d * 512:(d + 1) * 512],
                            start=(d == 0), stop=(d == DC - 1))
                    nc.vector.tensor_copy(k_raw[ti][:], pk[:, 0:H])
                    nc.scalar.copy(V[ti][:, 0:H], pk[:, H:2 * H])
                    junk = p1c.tile([128, H], F32, tag="junk", name=f"jk{ti}")
                    nc.vector.tensor_mul(junk[:], k_raw[ti][:], k_raw[ti][:])
                    nc.vector.reduce_sum(ssqk[:, ti:ti + 1], junk[:],
                                         axis=mybir.AxisListType.X)
                    if ti % 4 == 3:
                        rms_quarter(ssqk, rmsk, rinvk, ti // 4)
                    if ti >= 4:
                        tl = ti - 4
                        rope_pass(k_raw[tl], rinvk[:, tl:tl + 1], KT, tl,
                                  kw_sb if apply_w else None, f"k{tl}")
                for tl in range(TT - 4, TT):
                    rope_pass(k_raw[tl], rinvk[:, tl:tl + 1], KT, tl,
                              kw_sb if apply_w else None, f"k{tl}")

                # ---- per-head q pass; RoPE of head h overlaps head h+1 ----
                for h in range(HEADS_PC):
                    if h >= 2:
                        load_wq(h)
                    q_raw = [praw.tile([128, H], BF, tag=f"qr{ti}", bufs=1,
                                       name=f"qr{h}_{ti}") for ti in range(TT)]
                    ssqq = praw.tile([128, TT], F32, tag="ssqq", bufs=2,
                                     name=f"ssqq{h}")
                    rmsq = praw.tile([128, TT], F32, tag="rmsq", bufs=2,
                                     name=f"rmsq{h}")
                    rinvq = praw.tile([128, TT], F32, tag="rinvq", bufs=2,
                                      name=f"rinvq{h}")
                    last_h = (h == HEADS_PC - 1)
                    for ti in range(TT):
                        pq = psum.tile([128, 2 * H], F32, tag="big", bufs=4,
                                       name=f"pq{h}_{ti}")
                        for d in range(DC):
                            nc.tensor.matmul(
                                pq[:], xT_sb[:, xcol(d, ti):xcol(d, ti) + 128],
                                wq_sb[h][:, d * 512:(d + 1) * 512],
                                start=(d == 0), stop=(d == DC - 1))
                        nc.vector.tensor_copy(q_raw[ti][:], pq[:, 0:H])
                        nc.scalar.activation(gate[h][ti][:], pq[:, H:2 * H],
                                             Act.Sigmoid)
                        junk = p1c.tile([128, H], F32, tag="junk",
                                        name=f"jq{h}_{ti}")
                        nc.vector.tensor_mul(junk[:], q_raw[ti][:],
                                             q_raw[ti][:])
                        nc.vector.reduce_sum(ssqq[:, ti:ti + 1], junk[:],
                                             axis=mybir.AxisListType.X)
                        if last_h and ti >= 12:
                            # per-tile rms so the final rope tail collapses
                            nc.scalar.activation(rmsq[:, ti:ti + 1],
                                                 ssqq[:, ti:ti + 1], Act.Sqrt,
                                                 scale=1.0 / H, bias=epst[:])
                            nc.vector.reciprocal(rinvq[:, ti:ti + 1],
                                                 rmsq[:, ti:ti + 1])
                        elif ti % 4 == 3:
                            rms_quarter(ssqq, rmsq, rinvq, ti // 4)
                        ropes = []
                        if ti >= 4:
                            ropes.append(ti - 4)
                        if last_h and ti >= 13:
                            ropes.append(ti - 1)    # tiles 12..14 early
                        for tl in ropes:
                            rope_pass(q_raw[tl], rinvq[:, tl:tl + 1], QT[h],
                                      tl, qw_sb if apply_w else None,
                                      f"q{h}_{tl}")
                    tail = [15] if last_h else list(range(12, 16))
                    for tl in tail:
                        rope_pass(q_raw[tl], rinvq[:, tl:tl + 1], QT[h], tl,
                                  qw_sb if apply_w else None, f"q{h}_{tl}")

            # ================= phases 2+3 =================
            with tc.tile_pool(name="p23", bufs=1) as p23, \
                 tc.tile_pool(name="p2w", bufs=3) as p2w:
                qkvgT = [p23.tile([128, T], BF, tag=f"qkT{c}", name=f"qkT{c}")
                         for c in range(2 * HEADS_PC)]
                wo_sb = p23.tile([128, 8 * 2048], BF, tag="wo", name="wo_sb")
                for q_ in range(4):
                    nc.sync.dma_start(wo_sb[:, q_ * 4096:(q_ + 1) * 4096],
                                      wo[:, q_ * 4096:(q_ + 1) * 4096])

                def emit_po(ti):
                    """Phase-3 o-proj partial for one token tile."""
                    ot = p2w.tile([128, 2048], BF, tag="ot", bufs=2,
                                  name=f"ot{ti}")
                    for db in range(4):
                        po = psum.tile([128, 512], F32, tag="big", bufs=4,
                                       name=f"po{ti}_{db}")
                        for c in range(2 * HEADS_PC):
                            nc.tensor.matmul(
                                po[:], qkvgT[c][:, ti * 128:(ti + 1) * 128],
                                wo_sb[:, c * 2048 + db * 512:
                                      c * 2048 + (db + 1) * 512],
                                start=(c == 0), stop=(c == 2 * HEADS_PC - 1))
                        if db % 2 == 0:
                            nc.scalar.copy(ot[:, db * 512:(db + 1) * 512],
                                           po[:])
                        else:
                            nc.vector.tensor_copy(
                                ot[:, db * 512:(db + 1) * 512], po[:])
                    nc.sync.dma_start(out[ti * 128:(ti + 1) * 128, :], ot[:])

                tps_prev = None         # deferred PE transposes of last flush
                prev_j = None           # most recently completed query block
                for j in (3, 2, 1, 0):          # q blocks of 512, deep first
                    kmax = 4 * (j + 1) if causal else TT
                    for h in range(HEADS_PC):
                        # scores + exp for key block i (narrowed on diagonal)
                        def issue_st(i, nm, h=h, j=j):
                            dlt = max(0, i - 4 * j) if causal else 0
                            w = 512 - 128 * dlt
                            st = psum.tile([128, 512], F32, tag="big", bufs=4,
                                           name=f"st{nm}")
                            q0 = j * 512 + dlt * 128
                            for c2 in range(2):
                                nc.tensor.matmul(
                                    st[:, 0:w],
                                    KT[c2][:, i * 128:(i + 1) * 128],
                                    QT[h][c2][:, q0:(j + 1) * 512],
                                    start=(c2 == 0), stop=(c2 == 1))
                            if mode == "arbitrary":
                                amt = p2w.tile([128, 512], F32, tag="amt",
                                               name=f"am{nm}")
                                nc.sync.dma_start(
                                    amt[:], am[i * 128:(i + 1) * 128,
                                               j * 512:(j + 1) * 512])
                                nc.vector.tensor_add(st[:], st[:], amt[:])
                            pT = p2w.tile([128, 512], BF, tag="pT",
                                          name=f"pT{nm}")
                            nc.scalar.activation(pT[:, 0:w], st[:, 0:w],
                                                 Act.Exp, scale=SCALE)
                            if causal and i >= 4 * j:
                                nc.gpsimd.affine_select(
                                    out=pT[:, 0:128], in_=pT[:, 0:128],
                                    compare_op=Alu.is_ge, fill=0.0,
                                    base=0, channel_multiplier=-1,
                                    pattern=[[1, 128]])
                            return pT, dlt

                        # prologue scores, then deferred transposes of the
                        # previous head's flush (their DVE inputs are ready)
                        pts = {}
                        for i in range(min(PIPE, kmax)):
                            pts[i] = issue_st(i, f"{h}_{j}_{i}")
                        if tps_prev is not None:
                            tps_prev()
                            tps_prev = None
                        av = [psum.tile([128, H + 1], F32, tag=f"av{s}",
                                        name=f"av{h}_{j}_{s}")
                              for s in range(4)]
                        for i in range(kmax):
                            if i + PIPE < kmax:
                                pts[i + PIPE] = issue_st(i + PIPE,
                                                         f"{h}_{j}_{i+PIPE}")
                            pT, dlt = pts.pop(i)
                            for s in range(dlt, 4):
                                last_i = (4 * j + s) if causal else (kmax - 1)
                                if i > last_i:
                                    continue
                                nc.tensor.matmul(
                                    av[s][:],
                                    pT[:, (s - dlt) * 128:(s - dlt + 1) * 128],
                                    V[i][:], start=(i == 0),
                                    stop=(i == last_i))

                        # phase 3 with one-block lag keeps PE busy here
                        if prev_j is not None:
                            emit_po(4 * prev_j + h)

                        # flush DVE chain now (runs under the o-proj);
                        # PE transposes deferred past next head's prologue
                        qks = []
                        for s in range(4):
                            ti = 4 * j + s
                            rec = p2w.tile([128, 1], F32, tag="rec",
                                           name=f"rec{h}_{ti}")
                            if mode == "arbitrary":
                                dcl = p2w.tile([128, 1], F32, tag="dcl",
                                               name=f"dcl{h}_{ti}")
                                nc.vector.tensor_scalar_max(
                                    dcl[:], av[s][:, H:H + 1], 1e-30)
                                nc.vector.reciprocal(rec[:], dcl[:])
                            else:
                                nc.vector.reciprocal(rec[:], av[s][:, H:H + 1])
                            qk0 = p2w.tile([128, H], BF, tag="qk0",
                                           name=f"qk0{h}_{ti}")
                            nc.vector.tensor_mul(qk0[:], av[s][:, 0:H],
                                                 gate[h][ti][:])
                            qk = p2w.tile([128, H], BF, tag="qkg",
                                          name=f"qkg{h}_{ti}")
                            nc.vector.tensor_scalar_mul(qk[:], qk0[:], rec[:])
                            qks.append(qk)

                        def tps(qks=qks, h=h, j=j):
                            for s in range(4):
                                ti = 4 * j + s
                                for c2 in range(2):
                                    transpose_128(
                                        qks[s][:, c2 * 128:(c2 + 1) * 128],
                                        qkvgT[2 * h + c2][
                                            :, ti * 128:(ti + 1) * 128],
                                        f"o{h}_{ti}_{c2}")
                        tps_prev = tps
                    prev_j = j

                tps_prev()
                # trailing o-proj for the last query block processed
                for ti in range( 4 * prev_j, 4 * prev_j + 4):
                    emit_po(ti)

    nc.compile()
    return nc


def _get_compiled(mode, apply_w):
    key = (mode, apply_w)
    if key not in _COMPILED:
        _COMPILED[key] = _build(mode, apply_w)
    return _COMPILED[key]


def _rope_tables(positions):
    """Host: exact reference mRoPE sin/cos tables -> CC=[cos|cos], SS=[-sin|sin]."""
    fraction = 2.0 * np.arange(FREQ, dtype=np.float32) / ROTARY
    timescale = (ROPE_THETA ** fraction).astype(np.float32)
    CC, SS = [], []
    for b in range(positions.shape[1]):
        sinusoid = positions[:, b, :, None].astype(np.float32) / timescale
        freq = sinusoid[0].copy()
        h_idx = np.arange(1, 11 * 3, 3)
        w_idx = np.arange(2, 10 * 3, 3)
        freq[:, h_idx] = sinusoid[1][:, h_idx]
        freq[:, w_idx] = sinusoid[2][:, w_idx]
        sin, cos = np.sin(freq), np.cos(freq)
        CC.append(np.concatenate([cos, cos], axis=1).astype(np.float32))
        SS.append(np.concatenate([-sin, sin], axis=1).astype(np.float32))
    return CC, SS


def _pack_rows(arr, blk=128):
    """[R, C] with R = n*128 -> [128, n*C] (row-chunk-major columns)."""
    r, c = arr.shape
    n = r // blk
    return np.ascontiguousarray(
        arr.reshape(n, blk, c).transpose(1, 0, 2).reshape(blk, n * c))


def _pack_xT(xt):
    """[D, T] -> [128, 32768] with col = ti*2048 + d*128 + t."""
    # (16d, 128p, 16ti, 128t) -> (128p, 16ti, 16d, 128t)
    v = xt.reshape(DC, 128, TT, 128).transpose(1, 2, 0, 3)
    return np.ascontiguousarray(v.reshape(128, DC * 2048))


def kernel(x, positions, attn_mask, wq, wk, wv, wo, q_norm_w, k_norm_w):
    global LAST_RESULT, LAST_IN_MAPS
    x = np.asarray(x)
    positions = np.asarray(positions)
    attn_mask = np.asarray(attn_mask)
    wq, wk, wv, wo = map(np.asarray, (wq, wk, wv, wo))
    q_norm_w, k_norm_w = np.asarray(q_norm_w), np.asarray(k_norm_w)

    tril = np.tril(np.ones((T, T), dtype=bool))
    if all(np.array_equal(attn_mask[b], tril) for b in range(B)):
        mode = "causal"
    elif attn_mask.all():
        mode = "full"
    else:
        mode = "arbitrary"
    apply_w = bool(np.any(q_norm_w != 0) or np.any(k_norm_w != 0))

    nc = _get_compiled(mode, apply_w)
    CC, SS = _rope_tables(positions)
    group = N_HEADS // N_KV  # q heads per kv head = 8

    in_maps = []
    for c in range(8):
        b, g = c // 4, c % 4
        kvh = (g * HEADS_PC) // group
        wq_g = wq[:, g * HEADS_PC:(g + 1) * HEADS_PC, :]    # (D, 4, 512)
        wq_packed = np.concatenate(
            [_pack_rows(np.ascontiguousarray(wq_g[:, h, :]))
             for h in range(HEADS_PC)], axis=1)             # (128, 32768)
        m = {
            "xT": _pack_xT(np.ascontiguousarray(x[b].T)).astype(BF16),
            "wq": wq_packed.astype(BF16),
            "wkv": _pack_rows(np.concatenate(
                [wk[:, kvh, :], wv[:, kvh, :]], axis=1)).astype(BF16),
            "wo": _pack_rows(
                wo[g * HEADS_PC:(g + 1) * HEADS_PC].reshape(
                    HEADS_PC * H, D)).astype(BF16),
            "cc": _pack_rows(CC[b]).astype(BF16),
            "ss": _pack_rows(SS[b]).astype(BF16),
        }
        if apply_w:
            m["qw"] = np.ascontiguousarray(np.broadcast_to(
                (1.0 + q_norm_w).astype(np.float32), (128, H)))
            m["kw"] = np.ascontiguousarray(np.broadcast_to(
                (1.0 + k_norm_w).astype(np.float32), (128, H)))
        if mode == "arbitrary":
            m["am"] = np.where(attn_mask[b], np.float32(0.0),
                               np.float32(K_MASK)).astype(np.float32)
        in_maps.append(m)

    res = bass_utils.run_bass_kernel_spmd(nc, in_maps, core_ids=list(range(8)))
    LAST_RESULT = res
    LAST_IN_MAPS = in_maps
    out = np.zeros((B, T, D), np.float32)
    for c in range(8):
        out[c // 4] += res.results[c]["out"].astype(np.float32)
    return out
